# revision 2
# baseline (speedup 1.0000x reference)
"""Trainium2 kernel for the sum-product knowledge layer network.

Strategy:
  - Transposed on-chip layout: batch columns live on partitions (2 cols
    packed as an fp16 pair inside one f32 word), table rows along the
    free dim.  All gathers run on the GPSIMD (Pool) engine via ap_gather
    (SBUF->SBUF), eliminating nearly all DMA traffic.
  - Layer fusion: (product L0 + logsumexp L1) and (product L2 + logsumexp
    L3) are fused, so only one intermediate table (T2) is ever
    materialized.  Each fused layer gathers 8 source rows per output
    group (two fanin-4 sums -> one logsumexp pair).
  - Dead-row elimination: only T2 rows actually referenced by the final
    fused layer are computed (backward liveness through the pointers).
  - 4-way split: partitions are grouped [e*32+w] (e = group, w =
    col-word).  Each 16-partition gpsimd core gathers its own index
    list, so the 4 e-groups process disjoint quarters of each fused
    layer's groups concurrently (ap_gather cost is per-partition free
    size).  The encode is split the same way (each e-group computes a
    quarter of the vars).  Tables use an e-group-local-first layout and
    are re-assembled with 6 rotation DMA copies (the local quarter
    needs no copy).
  - Reductions on DVE in fp16 (2x mode) using wide strided ops; exp/log
    on ACT ordered to minimize activation-table reloads; final output
    f32.
"""

import numpy as np

P = 128
NCORES = 8
N_VARS = 4096
BATCH = 512
COLS = BATCH // NCORES     # 64 batch cols per core
NW = COLS // 2             # 32 col-words per e-group
NEG = 4                    # e-groups
NVQ = N_VARS // 4          # vars per e-group quarter
W0 = 2 * NVQ + 4           # T0 quarter width: 2048 rows + 4 zero-pad rows
OUT_SIZES = [16384, 8192, 4096, 2048]
N_OUT = OUT_SIZES[3]


def _round_up(x, m):
    return (x + m - 1) // m * m


def plan_from_ptrs(ptrs_list):
    """Backward liveness + fused-layer e-group-local-first index lists."""
    p0 = np.asarray(ptrs_list[0]).reshape(OUT_SIZES[0], 4)
    p1 = np.asarray(ptrs_list[1]).reshape(OUT_SIZES[1], 2)
    p2 = np.asarray(ptrs_list[2]).reshape(OUT_SIZES[2], 4)
    p3 = np.asarray(ptrs_list[3]).reshape(OUT_SIZES[3], 2)

    e23 = p2[p3]                      # [2048, 2, 4] -> T2 rows
    live2 = np.unique(e23)            # live T2 rows (sorted)
    e01 = p0[p1[live2]]               # [G2, 2, 4] -> T0 rows

    G2 = len(live2)
    cq2 = -(-G2 // 4)
    Gq2 = _round_up(cq2, 4)           # sub-chunks keep 8*n % 16 == 0
    Gq3 = N_OUT // 4                  # 512
    S01 = _round_up(int(Gq2 * 0.70), 4)  # uneven split: big first chunk

    plan = {"Gq2": Gq2, "cq2": cq2, "G2": G2, "Gq3": Gq3, "S01": S01}

    def build_idx(edges, G, cq, Gq, row_map, span=None):
        """k-major index list per e-group quarter; span=(gl, gh) restricts
        to that slice of each quarter's groups."""
        f = edges.shape[1]            # 8
        gl, gh = (0, Gq) if span is None else span
        Gqh = gh - gl
        ni = f * Gqh
        idx = np.zeros((8, ni), dtype=np.int16)
        for e in range(4):
            qend = min((e + 1) * cq, G)
            lo = min(e * cq + gl, qend)
            hi = min(e * cq + gh, qend)
            n = hi - lo
            lst = np.zeros((f, Gqh), dtype=np.int64)
            if n > 0:
                lst[:, :n] = row_map(edges[lo:hi].T, e)
            assert lst.max() < 2 ** 15 and lst.min() >= 0
            idx[2 * e] = idx[2 * e + 1] = lst.reshape(-1).astype(np.int16)
        return np.ascontiguousarray(
            idx.reshape(8, ni // 16, 16).transpose(0, 2, 1).reshape(P, ni // 16)
        )

    # L01 edges reference T0 rows (0..8193).  T0 device layout is
    # e-local-first quarters of width W0, each split into a pos block
    # [0, NVQ), a neg block [NVQ, 2*NVQ), and zero pads [2*NVQ, W0):
    # var i -> quarter i//NVQ, within-slot i%NVQ + NVQ*parity.
    def map01(rows, e):
        rows = np.asarray(rows)
        i = (rows - 2) // 2
        parity = rows & 1
        qv, wi = i // NVQ, i % NVQ
        out = W0 * ((qv - e) % 4) + wi + NVQ * parity
        return np.where(rows >= 2, out, 2 * NVQ)  # row 0/1 -> local zero pad

    plan["idx01a"] = build_idx(
        e01.reshape(G2, 8), G2, cq2, Gq2, map01, span=(0, S01)
    )
    plan["idx01b"] = build_idx(
        e01.reshape(G2, 8), G2, cq2, Gq2, map01, span=(S01, Gq2)
    )

    def map23(rows, e):
        pos = np.searchsorted(live2, rows)
        q, r = pos // cq2, pos % cq2
        return r + Gq2 * ((q - e) % 4)

    plan["idx23"] = build_idx(e23.reshape(N_OUT, 8), N_OUT, Gq3, Gq3, map23)
    return plan


def build_nc(Gq2, Gq3, S01):
    import concourse.bacc as bacc
    import concourse.mybir as mybir
    import concourse.tile as tile

    f32 = mybir.dt.float32
    f16 = mybir.dt.float16
    i16 = mybir.dt.int16
    Alu = mybir.AluOpType
    Act = mybir.ActivationFunctionType

    spans01 = [(0, S01), (S01, Gq2)]
    nihs = [8 * (gh - gl) for gl, gh in spans01]
    ni23 = 8 * Gq3
    nc = bacc.Bacc("TRN2", target_bir_lowering=False, debug=False)
    xq = nc.dram_tensor("xq", [P, NVQ * 2], f16, kind="ExternalInput")
    idx01_d = [
        nc.dram_tensor(f"idx01{s}", [P, nihs[i] // 16], i16, kind="ExternalInput")
        for i, s in enumerate("ab")
    ]
    idx23_d = nc.dram_tensor("idx23", [P, ni23 // 16], i16, kind="ExternalInput")
    out_d = nc.dram_tensor("out", [P, Gq3 * 2], f32, kind="ExternalOutput")

    with tile.TileContext(nc) as tc:
        with (
            tc.tile_pool(name="tabs", bufs=1) as tabs,
            tc.tile_pool(name="ix", bufs=1) as ixp,
            tc.tile_pool(name="xin", bufs=1) as xinp,
            tc.tile_pool(name="g", bufs=1) as gp,
            tc.tile_pool(name="tmp", bufs=1) as tp,
        ):
            T0 = tabs.tile([P, 4 * W0], f32, name="T0", tag="T0")
            T2 = tabs.tile([P, 4 * Gq2], f32, name="T2", tag="T2")

            def fixup(dst_tile, Gq, lo, hi):
                """Rotation copies: foreign slot j of partition p pulls
                local rows [lo:hi] from partition (p + 32j) % 128."""
                for j in range(1, 4):
                    s = 32 * j
                    dv = dst_tile[:]
                    nc.sync.dma_start(
                        dv[0 : P - s, j * Gq + lo : j * Gq + hi], dv[s:P, lo:hi]
                    )
                    nc.sync.dma_start(
                        dv[P - s : P, j * Gq + lo : j * Gq + hi], dv[0:s, lo:hi]
                    )

            # ---- encode (quartered): partition 32e+w computes vars
            # [NVQ*e, NVQ*(e+1)) for cols (2w, 2w+1) into T0's local
            # quarter [0, W0): slot 2i+parity; pad slots [2*NVQ, W0) = 0.
            xt = xinp.tile([P, NVQ, 2], f16, tag="xt")
            nc.sync.dma_start(
                xt[:], xq[:].rearrange("p (r q) -> p r q", q=2)
            )
            ix01 = []
            for s in range(2):
                t = ixp.tile(
                    [P, nihs[s] // 16], i16, tag=f"ix01{s}", name=f"ix01{s}"
                )
                nc.sync.dma_start(t[:], idx01_d[s][:])
                ix01.append(t)
            ix23 = ixp.tile([P, ni23 // 16], i16, tag="ix23")
            nc.sync.dma_start(ix23[:], idx23_d[:])

            # pos block: straight copy, then its fixup overlaps the ACT
            # exp/ln chain; neg block + zero pads fixed up after.
            t0h = T0[:].bitcast(f16)
            nc.vector.memset(T0[:][:, 2 * NVQ : W0], 0.0)
            nc.vector.tensor_copy(
                t0h[:, 0 : 2 * NVQ].rearrange("p (r q) -> p r q", q=2), xt[:]
            )
            fixup(T0, W0, 0, NVQ)
            et = xinp.tile([P, NVQ, 2], f32, tag="et")
            nc.scalar.activation(et[:], xt[:], Act.Exp)
            nc.scalar.activation(
                t0h[:, 2 * NVQ : 4 * NVQ].rearrange("p (r q) -> p r q", q=2),
                et[:], Act.Ln, scale=-1.0, bias=1.0,
            )
            fixup(T0, W0, NVQ, W0)

            def stage1(v, n2, hx):
                """v: gathered fp16 view [p, 8, n2]; returns (m, dd)."""
                u = tp.tile([P, 4, n2], f16, tag=f"u_{hx}", name="u")
                w = tp.tile([P, 2, n2], f16, tag=f"w_{hx}", name="w")
                m = tp.tile([P, n2], f16, tag=f"m_{hx}", name="m")
                dd = tp.tile([P, n2], f16, tag=f"dd_{hx}", name="dd")
                vv = v.rearrange("p (a b) n -> p a b n", b=2)
                nc.vector.tensor_add(u[:], vv[:, :, 0, :], vv[:, :, 1, :])
                uu = u[:].rearrange("p (a b) n -> p a b n", b=2)
                nc.vector.tensor_add(w[:], uu[:, :, 0, :], uu[:, :, 1, :])
                nc.vector.tensor_tensor(dd[:], w[:][:, 0, :], w[:][:, 1, :],
                                        op=Alu.min)
                nc.vector.tensor_tensor(m[:], w[:][:, 0, :], w[:][:, 1, :],
                                        op=Alu.max)
                nc.vector.tensor_tensor(dd[:], dd[:], m[:], op=Alu.subtract)
                return m, dd

            # ---- fused L0+L1: two chunked gathers pipelined with reduce.
            # Cross-engine waits are program-order-conservative (completion
            # counters), so each consumer is emitted right after its
            # producer; h1's final add is slotted between h2's DVE ops.
            gs, vs = [], []
            for hx in range(2):
                g = gp.tile([P, nihs[hx]], f32, tag=f"g01{hx}", name=f"g01{hx}")
                gs.append(g)
                vs.append(g[:].bitcast(f16).rearrange("p (k n) -> p k n", k=8))

            gl0, gh0 = spans01[0]
            gl1, gh1 = spans01[1]
            nc.gpsimd.ap_gather(gs[0][:], T0[:], ix01[0][:], P, 4 * W0, 1, nihs[0])
            nc.gpsimd.ap_gather(gs[1][:], T0[:], ix01[1][:], P, 4 * W0, 1, nihs[1])
            m1, dd1 = stage1(vs[0], 2 * (gh0 - gl0), 0)
            nc.scalar.activation(dd1[:], dd1[:], Act.Exp)
            nc.scalar.activation(dd1[:], dd1[:], Act.Ln, bias=1.0)
            # h2 stage1 begins; h1's add interleaves once ln1 lands
            n2b = 2 * (gh1 - gl1)
            u = tp.tile([P, 4, n2b], f16, tag="u_1", name="u")
            w = tp.tile([P, 2, n2b], f16, tag="w_1", name="w")
            m2 = tp.tile([P, n2b], f16, tag="m_1", name="m")
            dd2 = tp.tile([P, n2b], f16, tag="dd_1", name="dd")
            vv = vs[1].rearrange("p (a b) n -> p a b n", b=2)
            nc.vector.tensor_add(u[:], vv[:, :, 0, :], vv[:, :, 1, :])
            h1_out = T2[:].bitcast(f16)[:, 2 * gl0 : 2 * gh0]
            nc.vector.tensor_add(h1_out, m1[:], dd1[:])
            fixup(T2, Gq2, gl0, gh0)
            uu = u[:].rearrange("p (a b) n -> p a b n", b=2)
            nc.vector.tensor_add(w[:], uu[:, :, 0, :], uu[:, :, 1, :])
            nc.vector.tensor_tensor(dd2[:], w[:][:, 0, :], w[:][:, 1, :],
                                    op=Alu.min)
            nc.vector.tensor_tensor(m2[:], w[:][:, 0, :], w[:][:, 1, :],
                                    op=Alu.max)
            nc.vector.tensor_tensor(dd2[:], dd2[:], m2[:], op=Alu.subtract)
            nc.scalar.activation(dd2[:], dd2[:], Act.Exp)
            nc.scalar.activation(dd2[:], dd2[:], Act.Ln, bias=1.0)
            h2_out = T2[:].bitcast(f16)[:, 2 * gl1 : 2 * gh1]
            nc.vector.tensor_add(h2_out, m2[:], dd2[:])
            fixup(T2, Gq2, gl1, gh1)

            # prefetch the Exp act table while G23 runs (ACT is idle)
            warm = tp.tile([P, 4], f16, tag="warm")
            nc.scalar.activation(warm[:], warm[:], Act.Exp)

            # ---- fused L2+L3 (single gather, two reduce halves)
            g23 = gp.tile([P, ni23], f32, tag="g23", name="g23")
            nc.gpsimd.ap_gather(g23[:], T2[:], ix23[:], P, 4 * Gq2, 1, ni23)
            v23 = g23[:].bitcast(f16).rearrange("p (k n) -> p k n", k=8)
            ms, dds = [], []
            for hx, (lo, hi) in enumerate([(0, Gq3 // 2), (Gq3 // 2, Gq3)]):
                m, dd = stage1(v23[:, :, 2 * lo : 2 * hi], 2 * (hi - lo), 2 + hx)
                ms.append(m)
                dds.append(dd)
            for dd in dds:
                nc.scalar.activation(dd[:], dd[:], Act.Exp)
            for dd in dds:
                nc.scalar.activation(dd[:], dd[:], Act.Ln, bias=1.0)
            for hx, (lo, hi) in enumerate([(0, Gq3 // 2), (Gq3 // 2, Gq3)]):
                el, eh = 2 * lo, 2 * hi
                outt = tp.tile([P, eh - el], f32, tag=f"outt_{hx}", name="outt")
                nc.vector.tensor_add(outt[:], ms[hx], dds[hx][:])
                nc.sync.dma_start(out_d[:][:, el:eh], outt[:])
    nc.compile()
    return nc


def host_prep(x, ptrs_list):
    x = np.asarray(x, dtype=np.float32)
    plan = plan_from_ptrs(ptrs_list)
    in_maps = []
    for c in range(NCORES):
        xs = x[:, c * COLS : (c + 1) * COLS].astype(np.float16)  # [4096, 64]
        # partition 32e+w <- vars [NVQ*e, NVQ*(e+1)), cols (2w, 2w+1)
        xw = xs.reshape(NEG, NVQ, NW, 2).transpose(0, 2, 1, 3)  # [e, w, r, q]
        in_maps.append(
            {
                "xq": np.ascontiguousarray(xw.reshape(P, NVQ * 2)),
                "idx01a": plan["idx01a"],
                "idx01b": plan["idx01b"],
                "idx23": plan["idx23"],
            }
        )
    return plan, in_maps


def unscramble(outs, plan):
    """outs: per-core [128, 2*Gq3] f32 -> full [2048, 512] f32."""
    Gq = plan["Gq3"]
    full = np.empty((N_OUT, BATCH), dtype=np.float32)
    for c, o in enumerate(outs):
        o = o.reshape(NEG, NW, Gq, 2)  # [e, w, r, q]
        for e in range(4):
            lo = e * Gq
            full[lo : lo + Gq, c * COLS : (c + 1) * COLS] = (
                o[e].transpose(1, 0, 2).reshape(Gq, COLS)
            )
    return full


_CACHE = {}


def kernel(x, ptrs0, seg0, ptrs1, seg1, ptrs2, seg2, ptrs3, seg3):
    from concourse.bass_utils import run_bass_kernel_spmd

    ptrs_list = [ptrs0, ptrs1, ptrs2, ptrs3]
    plan, in_maps = host_prep(x, ptrs_list)
    key = (int(plan["Gq2"]), int(plan["Gq3"]), int(plan["S01"]))
    if key not in _CACHE:
        _CACHE[key] = build_nc(*key)
    nc = _CACHE[key]
    res = run_bass_kernel_spmd(nc, in_maps, core_ids=list(range(NCORES)))
    outs = [r["out"] for r in res.results]
    return unscramble(outs, plan)


# revision 3
# speedup vs baseline: 1.0016x; 1.0016x over previous
"""Trainium2 kernel for the sum-product knowledge layer network.

Strategy:
  - Transposed on-chip layout: batch columns live on partitions (2 cols
    packed as an fp16 pair inside one f32 word), table rows along the
    free dim.  All gathers run on the GPSIMD (Pool) engine via ap_gather
    (SBUF->SBUF), eliminating nearly all DMA traffic.
  - Layer fusion: (product L0 + logsumexp L1) and (product L2 + logsumexp
    L3) are fused, so only one intermediate table (T2) is ever
    materialized.  Each fused layer gathers 8 source rows per output
    group (two fanin-4 sums -> one logsumexp pair).
  - Dead-row elimination: only T2 rows actually referenced by the final
    fused layer are computed (backward liveness through the pointers).
  - 4-way split: partitions are grouped [e*32+w] (e = group, w =
    col-word).  Each 16-partition gpsimd core gathers its own index
    list, so the 4 e-groups process disjoint quarters of each fused
    layer's groups concurrently (ap_gather cost is per-partition free
    size).  The encode is split the same way (each e-group computes a
    quarter of the vars).  Tables use an e-group-local-first layout and
    are re-assembled with 6 rotation DMA copies (the local quarter
    needs no copy).
  - Reductions on DVE in fp16 (2x mode) using wide strided ops; exp/log
    on ACT ordered to minimize activation-table reloads; final output
    f32.
"""

import numpy as np

P = 128
NCORES = 8
N_VARS = 4096
BATCH = 512
COLS = BATCH // NCORES     # 64 batch cols per core
NW = COLS // 2             # 32 col-words per e-group
NEG = 4                    # e-groups
NVQ = N_VARS // 4          # vars per e-group quarter
W0 = 2 * NVQ + 4           # T0 quarter width: 2048 rows + 4 zero-pad rows
OUT_SIZES = [16384, 8192, 4096, 2048]
N_OUT = OUT_SIZES[3]


def _round_up(x, m):
    return (x + m - 1) // m * m


def plan_from_ptrs(ptrs_list):
    """Backward liveness + fused-layer e-group-local-first index lists."""
    p0 = np.asarray(ptrs_list[0]).reshape(OUT_SIZES[0], 4)
    p1 = np.asarray(ptrs_list[1]).reshape(OUT_SIZES[1], 2)
    p2 = np.asarray(ptrs_list[2]).reshape(OUT_SIZES[2], 4)
    p3 = np.asarray(ptrs_list[3]).reshape(OUT_SIZES[3], 2)

    e23 = p2[p3]                      # [2048, 2, 4] -> T2 rows
    live2 = np.unique(e23)            # live T2 rows (sorted)
    e01 = p0[p1[live2]]               # [G2, 2, 4] -> T0 rows

    G2 = len(live2)
    cq2 = -(-G2 // 4)
    Gq2 = _round_up(cq2, 4)           # sub-chunks keep 8*n % 16 == 0
    Gq3 = N_OUT // 4                  # 512
    S01 = _round_up(int(Gq2 * 0.70), 4)  # uneven split: big first chunk

    plan = {"Gq2": Gq2, "cq2": cq2, "G2": G2, "Gq3": Gq3, "S01": S01}

    def build_idx(edges, G, cq, Gq, row_map, span=None):
        """k-major index list per e-group quarter; span=(gl, gh) restricts
        to that slice of each quarter's groups."""
        f = edges.shape[1]            # 8
        gl, gh = (0, Gq) if span is None else span
        Gqh = gh - gl
        ni = f * Gqh
        idx = np.zeros((8, ni), dtype=np.int16)
        for e in range(4):
            qend = min((e + 1) * cq, G)
            lo = min(e * cq + gl, qend)
            hi = min(e * cq + gh, qend)
            n = hi - lo
            lst = np.zeros((f, Gqh), dtype=np.int64)
            if n > 0:
                lst[:, :n] = row_map(edges[lo:hi].T, e)
            assert lst.max() < 2 ** 15 and lst.min() >= 0
            idx[2 * e] = idx[2 * e + 1] = lst.reshape(-1).astype(np.int16)
        return np.ascontiguousarray(
            idx.reshape(8, ni // 16, 16).transpose(0, 2, 1).reshape(P, ni // 16)
        )

    # L01 edges reference T0 rows (0..8193).  T0 device layout is
    # e-local-first quarters of width W0, each split into a pos block
    # [0, NVQ), a neg block [NVQ, 2*NVQ), and zero pads [2*NVQ, W0):
    # var i -> quarter i//NVQ, within-slot i%NVQ + NVQ*parity.
    def map01(rows, e):
        rows = np.asarray(rows)
        i = (rows - 2) // 2
        parity = rows & 1
        qv, wi = i // NVQ, i % NVQ
        out = W0 * ((qv - e) % 4) + wi + NVQ * parity
        # row 1 (zeros) -> pad slot 2*NVQ; row 0 (-inf) -> pad slot 2*NVQ+1
        return np.where(rows >= 2, out, 2 * NVQ + (rows == 0))

    plan["idx01a"] = build_idx(
        e01.reshape(G2, 8), G2, cq2, Gq2, map01, span=(0, S01)
    )
    plan["idx01b"] = build_idx(
        e01.reshape(G2, 8), G2, cq2, Gq2, map01, span=(S01, Gq2)
    )

    def map23(rows, e):
        pos = np.searchsorted(live2, rows)
        q, r = pos // cq2, pos % cq2
        return r + Gq2 * ((q - e) % 4)

    plan["idx23"] = build_idx(e23.reshape(N_OUT, 8), N_OUT, Gq3, Gq3, map23)
    return plan


def build_nc(Gq2, Gq3, S01):
    import concourse.bacc as bacc
    import concourse.mybir as mybir
    import concourse.tile as tile

    f32 = mybir.dt.float32
    f16 = mybir.dt.float16
    i16 = mybir.dt.int16
    Alu = mybir.AluOpType
    Act = mybir.ActivationFunctionType

    spans01 = [(0, S01), (S01, Gq2)]
    nihs = [8 * (gh - gl) for gl, gh in spans01]
    ni23 = 8 * Gq3
    nc = bacc.Bacc("TRN2", target_bir_lowering=False, debug=False)
    xq = nc.dram_tensor("xq", [P, NVQ * 2], f16, kind="ExternalInput")
    idx01_d = [
        nc.dram_tensor(f"idx01{s}", [P, nihs[i] // 16], i16, kind="ExternalInput")
        for i, s in enumerate("ab")
    ]
    idx23_d = nc.dram_tensor("idx23", [P, ni23 // 16], i16, kind="ExternalInput")
    out_d = nc.dram_tensor("out", [P, Gq3 * 2], f32, kind="ExternalOutput")

    with tile.TileContext(nc) as tc:
        with (
            tc.tile_pool(name="tabs", bufs=1) as tabs,
            tc.tile_pool(name="ix", bufs=1) as ixp,
            tc.tile_pool(name="xin", bufs=1) as xinp,
            tc.tile_pool(name="g", bufs=1) as gp,
            tc.tile_pool(name="tmp", bufs=1) as tp,
        ):
            T0 = tabs.tile([P, 4 * W0], f32, name="T0", tag="T0")
            T2 = tabs.tile([P, 4 * Gq2], f32, name="T2", tag="T2")

            def fixup(dst_tile, Gq, lo, hi):
                """Rotation copies: foreign slot j of partition p pulls
                local rows [lo:hi] from partition (p + 32j) % 128."""
                for j in range(1, 4):
                    s = 32 * j
                    dv = dst_tile[:]
                    nc.sync.dma_start(
                        dv[0 : P - s, j * Gq + lo : j * Gq + hi], dv[s:P, lo:hi]
                    )
                    nc.sync.dma_start(
                        dv[P - s : P, j * Gq + lo : j * Gq + hi], dv[0:s, lo:hi]
                    )

            # ---- encode (quartered): partition 32e+w computes vars
            # [NVQ*e, NVQ*(e+1)) for cols (2w, 2w+1) into T0's local
            # quarter [0, W0): slot 2i+parity; pad slots [2*NVQ, W0) = 0.
            xt = xinp.tile([P, NVQ, 2], f16, tag="xt")
            nc.sync.dma_start(
                xt[:], xq[:].rearrange("p (r q) -> p r q", q=2)
            )
            ix01 = []
            for s in range(2):
                t = ixp.tile(
                    [P, nihs[s] // 16], i16, tag=f"ix01{s}", name=f"ix01{s}"
                )
                nc.sync.dma_start(t[:], idx01_d[s][:])
                ix01.append(t)
            ix23 = ixp.tile([P, ni23 // 16], i16, tag="ix23")
            nc.sync.dma_start(ix23[:], idx23_d[:])

            # pos block: straight copy, then its fixup overlaps the ACT
            # exp/ln chain; neg block + zero pads fixed up after.
            t0h = T0[:].bitcast(f16)
            nc.vector.memset(T0[:][:, 2 * NVQ : W0], 0.0)
            nc.vector.memset(
                t0h[:, 2 * (2 * NVQ + 1) : 2 * (2 * NVQ + 2)], float("-inf")
            )
            nc.vector.tensor_copy(
                t0h[:, 0 : 2 * NVQ].rearrange("p (r q) -> p r q", q=2), xt[:]
            )
            fixup(T0, W0, 0, NVQ)
            et = xinp.tile([P, NVQ, 2], f32, tag="et")
            nc.scalar.activation(et[:], xt[:], Act.Exp)
            nc.scalar.activation(
                t0h[:, 2 * NVQ : 4 * NVQ].rearrange("p (r q) -> p r q", q=2),
                et[:], Act.Ln, scale=-1.0, bias=1.0,
            )
            fixup(T0, W0, NVQ, W0)

            def stage1(v, n2, hx):
                """v: gathered fp16 view [p, 8, n2]; returns (m, dd)."""
                u = tp.tile([P, 4, n2], f16, tag=f"u_{hx}", name="u")
                w = tp.tile([P, 2, n2], f16, tag=f"w_{hx}", name="w")
                m = tp.tile([P, n2], f16, tag=f"m_{hx}", name="m")
                dd = tp.tile([P, n2], f16, tag=f"dd_{hx}", name="dd")
                vv = v.rearrange("p (a b) n -> p a b n", b=2)
                nc.vector.tensor_add(u[:], vv[:, :, 0, :], vv[:, :, 1, :])
                uu = u[:].rearrange("p (a b) n -> p a b n", b=2)
                nc.vector.tensor_add(w[:], uu[:, :, 0, :], uu[:, :, 1, :])
                nc.vector.tensor_tensor(dd[:], w[:][:, 0, :], w[:][:, 1, :],
                                        op=Alu.min)
                nc.vector.tensor_tensor(m[:], w[:][:, 0, :], w[:][:, 1, :],
                                        op=Alu.max)
                nc.vector.tensor_tensor(dd[:], dd[:], m[:], op=Alu.subtract)
                return m, dd

            # ---- fused L0+L1: two chunked gathers pipelined with reduce.
            # Cross-engine waits are program-order-conservative (completion
            # counters), so each consumer is emitted right after its
            # producer; h1's final add is slotted between h2's DVE ops.
            gs, vs = [], []
            for hx in range(2):
                g = gp.tile([P, nihs[hx]], f32, tag=f"g01{hx}", name=f"g01{hx}")
                gs.append(g)
                vs.append(g[:].bitcast(f16).rearrange("p (k n) -> p k n", k=8))

            gl0, gh0 = spans01[0]
            gl1, gh1 = spans01[1]
            nc.gpsimd.ap_gather(gs[0][:], T0[:], ix01[0][:], P, 4 * W0, 1, nihs[0])
            nc.gpsimd.ap_gather(gs[1][:], T0[:], ix01[1][:], P, 4 * W0, 1, nihs[1])
            m1, dd1 = stage1(vs[0], 2 * (gh0 - gl0), 0)
            nc.scalar.activation(dd1[:], dd1[:], Act.Exp)
            nc.scalar.activation(dd1[:], dd1[:], Act.Ln, bias=1.0)
            # h2 stage1 begins; h1's add interleaves once ln1 lands
            n2b = 2 * (gh1 - gl1)
            u = tp.tile([P, 4, n2b], f16, tag="u_1", name="u")
            w = tp.tile([P, 2, n2b], f16, tag="w_1", name="w")
            m2 = tp.tile([P, n2b], f16, tag="m_1", name="m")
            dd2 = tp.tile([P, n2b], f16, tag="dd_1", name="dd")
            vv = vs[1].rearrange("p (a b) n -> p a b n", b=2)
            nc.vector.tensor_add(u[:], vv[:, :, 0, :], vv[:, :, 1, :])
            h1_out = T2[:].bitcast(f16)[:, 2 * gl0 : 2 * gh0]
            nc.vector.tensor_add(h1_out, m1[:], dd1[:])
            fixup(T2, Gq2, gl0, gh0)
            uu = u[:].rearrange("p (a b) n -> p a b n", b=2)
            nc.vector.tensor_add(w[:], uu[:, :, 0, :], uu[:, :, 1, :])
            nc.vector.tensor_tensor(dd2[:], w[:][:, 0, :], w[:][:, 1, :],
                                    op=Alu.min)
            nc.vector.tensor_tensor(m2[:], w[:][:, 0, :], w[:][:, 1, :],
                                    op=Alu.max)
            nc.vector.tensor_tensor(dd2[:], dd2[:], m2[:], op=Alu.subtract)
            nc.scalar.activation(dd2[:], dd2[:], Act.Exp)
            nc.scalar.activation(dd2[:], dd2[:], Act.Ln, bias=1.0)
            h2_out = T2[:].bitcast(f16)[:, 2 * gl1 : 2 * gh1]
            nc.vector.tensor_add(h2_out, m2[:], dd2[:])
            fixup(T2, Gq2, gl1, gh1)

            # prefetch the Exp act table while G23 runs (ACT is idle)
            warm = tp.tile([P, 4], f16, tag="warm")
            nc.scalar.activation(warm[:], warm[:], Act.Exp)

            # ---- fused L2+L3 (single gather, two reduce halves)
            g23 = gp.tile([P, ni23], f32, tag="g23", name="g23")
            nc.gpsimd.ap_gather(g23[:], T2[:], ix23[:], P, 4 * Gq2, 1, ni23)
            v23 = g23[:].bitcast(f16).rearrange("p (k n) -> p k n", k=8)
            ms, dds = [], []
            for hx, (lo, hi) in enumerate([(0, Gq3 // 2), (Gq3 // 2, Gq3)]):
                m, dd = stage1(v23[:, :, 2 * lo : 2 * hi], 2 * (hi - lo), 2 + hx)
                ms.append(m)
                dds.append(dd)
            for dd in dds:
                nc.scalar.activation(dd[:], dd[:], Act.Exp)
            for dd in dds:
                nc.scalar.activation(dd[:], dd[:], Act.Ln, bias=1.0)
            for hx, (lo, hi) in enumerate([(0, Gq3 // 2), (Gq3 // 2, Gq3)]):
                el, eh = 2 * lo, 2 * hi
                outt = tp.tile([P, eh - el], f32, tag=f"outt_{hx}", name="outt")
                nc.vector.tensor_add(outt[:], ms[hx], dds[hx][:])
                nc.sync.dma_start(out_d[:][:, el:eh], outt[:])
    nc.compile()
    return nc


def host_prep(x, ptrs_list):
    x = np.asarray(x, dtype=np.float32)
    plan = plan_from_ptrs(ptrs_list)
    in_maps = []
    for c in range(NCORES):
        xs = x[:, c * COLS : (c + 1) * COLS].astype(np.float16)  # [4096, 64]
        # partition 32e+w <- vars [NVQ*e, NVQ*(e+1)), cols (2w, 2w+1)
        xw = xs.reshape(NEG, NVQ, NW, 2).transpose(0, 2, 1, 3)  # [e, w, r, q]
        in_maps.append(
            {
                "xq": np.ascontiguousarray(xw.reshape(P, NVQ * 2)),
                "idx01a": plan["idx01a"],
                "idx01b": plan["idx01b"],
                "idx23": plan["idx23"],
            }
        )
    return plan, in_maps


def unscramble(outs, plan):
    """outs: per-core [128, 2*Gq3] f32 -> full [2048, 512] f32."""
    Gq = plan["Gq3"]
    full = np.empty((N_OUT, BATCH), dtype=np.float32)
    for c, o in enumerate(outs):
        o = o.reshape(NEG, NW, Gq, 2)  # [e, w, r, q]
        for e in range(4):
            lo = e * Gq
            full[lo : lo + Gq, c * COLS : (c + 1) * COLS] = (
                o[e].transpose(1, 0, 2).reshape(Gq, COLS)
            )
    return full


_CACHE = {}


def kernel(x, ptrs0, seg0, ptrs1, seg1, ptrs2, seg2, ptrs3, seg3):
    from concourse.bass_utils import run_bass_kernel_spmd

    ptrs_list = [ptrs0, ptrs1, ptrs2, ptrs3]
    plan, in_maps = host_prep(x, ptrs_list)
    key = (int(plan["Gq2"]), int(plan["Gq3"]), int(plan["S01"]))
    if key not in _CACHE:
        _CACHE[key] = build_nc(*key)
    nc = _CACHE[key]
    res = run_bass_kernel_spmd(nc, in_maps, core_ids=list(range(NCORES)))
    outs = [r["out"] for r in res.results]
    return unscramble(outs, plan)
